# revision 13
# baseline (speedup 1.0000x reference)
"""Trainium2 Bass kernel for CanonCausalMultiheadAttn (v2).

Sharding: tensor-parallel over heads across 8 cores (2 q-heads + 1 kv-head
per core), both batches replicated. Attention outputs are exchanged with
TWO AllToAlls (one per local q-head) so round 0 overlaps round-1 compute;
each core then owns one (batch, 512-seq-slice) of the output projection.

Per-core pipeline (B=2, S=2048, D=2048 hardcoded):
  QKV proj: bf16 matmuls, stationary weight reused across 4 seq chunks,
  PE queue is pure matmuls (no norm/rope work) -> canon conv on full rows
  (DVE taps, first tap folded into the ACT psum-drain copy) -> qk rmsnorm
  with sum-of-squares on GPSIMD partition_all_reduce (PE-free) -> q rstd
  broadcast on GPSIMD, k rstd applied later as the EXP per-partition scale
  -> RoPE (DVE, bf16 tables with norm weight & 1/sqrt(dh) folded in) ->
  causal attention with scores in [Sk, Sq] layout; per K-block one wide
  matmul row + one wide EXP -> PV swapped (stationary = V block, moving =
  P) emitting O'^T [dh, q] directly in a2a layout, denominators via
  ones-column rowsum matmuls, normalization folded into the psum drain ->
  AllToAll x2 -> output projection (stationary = received attn block,
  resident Wo in SBUF).
"""
import sys

sys.path.insert(0, '/opt/trn_rl_repo')

import numpy as np
import ml_dtypes

import concourse.bass as bass
import concourse.mybir as mybir
import concourse.tile as tile
import concourse.bass_isa as bass_isa
from concourse import bacc
from concourse import library_config
from concourse.bass_utils import run_bass_kernel_spmd

F32 = mybir.dt.float32
F32R = mybir.dt.float32r
BF16 = mybir.dt.bfloat16
AF = mybir.ActivationFunctionType
ALU = mybir.AluOpType
RED = bass_isa.ReduceOp

B, S, D = 2, 2048, 2048
NH, NKV, DH = 16, 8, 128
K_CONV = 4
EPS = 1e-6
SCALE = 1.0 / float(np.sqrt(DH))
NEG = -1e9
N_CORES = 8
NCH = S // 512          # 512-wide seq chunks
NI = S // 128           # 128-wide Sk blocks


def _build():
    nc = bacc.Bacc("TRN2", target_bir_lowering=False, debug=False,
                   num_devices=N_CORES)

    hsT = nc.dram_tensor("hsT", [D, B * S], BF16, kind="ExternalInput")
    wT = nc.dram_tensor("wT", [D, 512], BF16, kind="ExternalInput")
    woT = nc.dram_tensor("woT", [D, D], BF16, kind="ExternalInput")
    cw = nc.dram_tensor("cw", [512, K_CONV], F32, kind="ExternalInput")
    ropeAq = nc.dram_tensor("ropeAq", [DH, S], BF16, kind="ExternalInput")
    ropeBq = nc.dram_tensor("ropeBq", [DH, S], BF16, kind="ExternalInput")
    ropeAk = nc.dram_tensor("ropeAk", [DH, S], BF16, kind="ExternalInput")
    ropeBk = nc.dram_tensor("ropeBk", [DH, S], BF16, kind="ExternalInput")
    maskd = nc.dram_tensor("maskd", [128, 128], F32, kind="ExternalInput")
    out = nc.dram_tensor("out", [512, D], F32, kind="ExternalOutput")

    with tile.TileContext(nc) as tc:
        nc.gpsimd.load_library(library_config.attn)
        with tc.tile_pool(name="const", bufs=1) as cpool, \
             tc.tile_pool(name="pers", bufs=1) as pers, \
             tc.tile_pool(name="dram", bufs=1, space="DRAM") as dram:

            # ---- small constants ----
            cw_sb = []
            for mt in range(4):
                t = cpool.tile([128, K_CONV], F32, tag=f"cw{mt}",
                               name=f"cw{mt}")
                nc.sync.dma_start(t[:], cw.ap()[128 * mt:128 * mt + 128, :])
                cw_sb.append(t)
            mask_sb = cpool.tile([128, 128], F32, tag="mask")
            nc.sync.dma_start(mask_sb[:], maskd.ap())
            ones_col_f = cpool.tile([128, 1], F32, tag="ocf")
            nc.vector.memset(ones_col_f[:], 1.0)
            ones_col = cpool.tile([128, 1], F32R, tag="oc")
            nc.scalar.copy(ones_col[:], ones_col_f[:])
            ones_row_f = cpool.tile([1, 128], F32, tag="orf")
            nc.vector.memset(ones_row_f[:], 1.0)
            ones_row = cpool.tile([1, 128], F32R, tag="or")
            nc.scalar.copy(ones_row[:], ones_row_f[:])
            ones_col_bf = cpool.tile([128, 1], BF16, tag="ocb")
            nc.scalar.copy(ones_col_bf[:], ones_col_f[:])
            eps_sb = cpool.tile([1, 1], F32, tag="eps")
            nc.vector.memset(eps_sb[:], EPS)
            s0_sb = []
            for mt in range(4):
                t = cpool.tile([128, 1], F32, tag=f"s0{mt}", name=f"s0{mt}")
                nc.vector.tensor_scalar_add(t[:], cw_sb[mt][:, 0:1], 1.0)
                s0_sb.append(t)

            # persistent per-batch tiles
            roped = {}   # (b, mt<3) -> [128, S] bf16   (q0, q1, kT)
            vaug = {}    # b -> [128, NI*128] bf16      (V transposed blocks)
            rstdkT = {}  # b -> [128, NI] f32

            for b in range(B):
                for mt in range(3):
                    roped[(b, mt)] = pers.tile([128, S], BF16,
                                               tag=f"roped{b}{mt}",
                                               name=f"roped{b}{mt}")
                vaug[b] = pers.tile([128, NI * 128], BF16, tag=f"vaug{b}",
                                    name=f"vaug{b}")
                rstdkT[b] = pers.tile([128, NI], F32, tag=f"rstdkT{b}",
                                      name=f"rstdkT{b}")

            rk_d = {b: dram.tile([NI, 128], F32, tag=f"rkd{b}",
                                 name=f"rk_d{b}") for b in range(B)}

            # rope tables (bf16), freed after prep
            with tc.tile_pool(name="ropes", bufs=1) as rpool:
                ropes = {}
                for nm, t in (("Aq", ropeAq), ("Bq", ropeBq),
                              ("Ak", ropeAk), ("Bk", ropeBk)):
                    rt = rpool.tile([DH, S], BF16, tag=f"rope{nm}",
                                    name=f"rope{nm}")
                    nc.sync.dma_start(rt[:], t.ap())
                    ropes[nm] = rt

                hs_sb = None
                for b in range(B):
                    # =============== QKV projection, batch b ===============
                    if hs_sb is None:
                        hs_cm = tc.tile_pool(name="hs", bufs=1)
                        hs_pool = hs_cm.__enter__()
                        hs_sb = hs_pool.tile([128, 16 * S], BF16, tag="hs",
                                             name="hs_sb")
                    hv = hs_sb[:].rearrange("p (k s) -> p k s", s=S)
                    for kg in range(4):
                        nc.sync.dma_start(
                            hv[:, 4 * kg:4 * kg + 4, :],
                            hsT.ap()[:, b * S:(b + 1) * S]
                            .rearrange("(k p) s -> p k s", p=128)
                            [:, 4 * kg:4 * kg + 4, :])

                    with tc.tile_pool(name=f"cn{b}", bufs=1) as bw, \
                         tc.tile_pool(name=f"qps{b}", bufs=2,
                                      space="PSUM") as qps:
                        cn = {}
                        for mt in range(3):
                            cn[mt] = bw.tile([128, S], F32, tag=f"cn{mt}",
                                             name=f"cn{mt}")
                        cn[3] = bw.tile([128, S], BF16, tag="cn3", name="cn3")

                        for mt in range(4):
                            psums = [qps.tile([128, 512], F32, tag=f"q{n}",
                                              name=f"q{n}")
                                     for n in range(NCH)]
                            for k in range(16):
                                wt_k = bw.tile([128, 128], BF16, tag="wtk",
                                               bufs=6, name="wt_k")
                                nc.sync.dma_start(
                                    wt_k[:],
                                    wT.ap()[128 * k:128 * (k + 1),
                                            128 * mt:128 * (mt + 1)])
                                for n in range(NCH):
                                    nc.tensor.matmul(
                                        psums[n][:], wt_k[:],
                                        hv[:, k, 512 * n:512 * (n + 1)],
                                        start=(k == 0), stop=(k == 15))
                            # drain psum -> raw (scale=1) and cn (scale=1+w0)
                            is_v = (mt == 3)
                            if is_v:
                                raw = bw.tile([128, S], BF16, tag="rawv",
                                              bufs=1, name="raw")
                            else:
                                raw = bw.tile([128, S], F32, tag="scr",
                                              bufs=2, name="raw")
                            c = cn[mt]
                            for n in range(NCH):
                                sl = slice(512 * n, 512 * (n + 1))
                                nc.scalar.copy(raw[:, sl], psums[n][:])
                                nc.scalar.activation(c[:, sl], psums[n][:],
                                                     AF.Copy,
                                                     scale=s0_sb[mt][:])
                            # canon taps on the full row (in-place)
                            for k in range(1, K_CONV):
                                nc.vector.scalar_tensor_tensor(
                                    c[:, k:S], raw[:, 0:S - k],
                                    cw_sb[mt][:, k:k + 1], c[:, k:S],
                                    ALU.mult, ALU.add)

                        # =============== prep (no PE ops), batch b =========
                        # V: transpose canon output into va blocks
                        va = vaug[b]
                        for i in range(NI):
                            nc.sync.dma_start_transpose(
                                va[:, 128 * i:128 * (i + 1)],
                                cn[3][:, 128 * i:128 * (i + 1)])

                        for mt in range(3):
                            x = cn[mt]
                            is_q = mt < 2
                            # sum of squares across partitions via GPSIMD
                            sq = bw.tile([128, S], F32, tag="scr", bufs=2,
                                         name="sq")
                            nc.vector.tensor_mul(sq[:], x[:], x[:])
                            red = bw.tile([128, S], F32, tag="scr", bufs=2,
                                          name="red")
                            nc.gpsimd.partition_all_reduce(
                                red[:], sq[:], channels=128,
                                reduce_op=RED.add)
                            srt = bw.tile([1, S], F32, tag="srt", bufs=1,
                                          name="srt")
                            nc.scalar.activation(srt[:], red[0:1, :],
                                                 AF.Sqrt, bias=eps_sb[:],
                                                 scale=1.0 / DH)
                            rstd = bw.tile([1, S], F32, tag="rstd", bufs=1,
                                           name="rstd")
                            nc.vector.reciprocal(rstd[:], srt[:])
                            bc = None
                            if is_q:
                                bc = bw.tile([128, S], F32, tag="bc",
                                             bufs=1, name="bc")
                                nc.gpsimd.partition_broadcast(
                                    bc[:], rstd[:], channels=128)
                            else:
                                nc.sync.dma_start(rk_d[b][:], rstd[:])
                                nc.sync.dma_start(
                                    rstdkT[b][:],
                                    rk_d[b][:].rearrange("i p -> p i"))
                            # rope
                            A_ = ropes["Aq"] if is_q else ropes["Ak"]
                            B_ = ropes["Bq"] if is_q else ropes["Bk"]
                            sh = bw.tile([128, S], F32, tag="sh", name="sh")
                            nc.sync.dma_start(sh[0:64, :], x[64:128, :])
                            nc.sync.dma_start(sh[64:128, :], x[0:64, :])
                            nc.vector.tensor_mul(sh[:], sh[:], B_[:])
                            tm = bw.tile([128, S], F32, tag="tm", name="tm")
                            nc.vector.tensor_mul(tm[:], x[:], A_[:])
                            ro = roped[(b, mt)]
                            if is_q:
                                nc.vector.tensor_add(tm[:], tm[:], sh[:])
                                nc.vector.tensor_mul(ro[:], tm[:], bc[:])
                            else:
                                nc.vector.tensor_add(ro[:], tm[:], sh[:])

                if b == 0:
                    # b1's hs DMAs reuse the same tile (WAR dep on QKV0)
                    pass
                if b == B - 1:
                    hs_cm.__exit__(None, None, None)

            # ======================= attention =======================
            a2a_in = [dram.tile([1024, 512], BF16, tag=f"a2ai{h}",
                                name=f"a2a_in{h}") for h in range(2)]
            a2a_out = [dram.tile([1024, 512], BF16, tag=f"a2ao{h}",
                                 name=f"a2a_out{h}") for h in range(2)]

            # resident Wo, prefetched during attention
            wo_cm = tc.tile_pool(name="wo", bufs=1)
            wo_pool = wo_cm.__enter__()
            wo_sb = wo_pool.tile([128, 16 * D], BF16, tag="wo",
                                 name="wo_sb")
            wov = wo_sb[:].rearrange("p (k c) -> p k c", c=D)
            for kg in range(4):
                nc.sync.dma_start(
                    wov[:, 4 * kg:4 * kg + 4, :],
                    woT.ap().rearrange("(k p) c -> p k c", p=128)
                    [:, 4 * kg:4 * kg + 4, :])

            for h in range(2):
                for b in range(B):
                    KT = roped[(b, 2)]
                    QT = roped[(b, h)]
                    va = vaug[b]
                    rkt = rstdkT[b]
                    with tc.tile_pool(name=f"pp{h}{b}", bufs=1) as ppool:
                        ptiles = []
                        # ---- phase A: scores + exp, per Sk block ----
                        with tc.tile_pool(name=f"sc{h}{b}", bufs=2,
                                          space="PSUM") as scps:
                            for i in range(NI):
                                lo = 128 * i
                                sc = scps.tile([128, 2048], F32, tag="sc",
                                               name="sc")
                                for n in range(lo // 512, NCH):
                                    c0 = max(lo, 512 * n)
                                    nc.tensor.matmul(
                                        sc[:, c0:512 * (n + 1)],
                                        KT[:, lo:lo + 128],
                                        QT[:, c0:512 * (n + 1)],
                                        start=True, stop=True)
                                nc.vector.tensor_add(
                                    sc[:, lo:lo + 128],
                                    sc[:, lo:lo + 128], mask_sb[:])
                                pt = ppool.tile([128, 2048], BF16,
                                                tag=f"p{i}", name=f"p{i}")
                                nc.scalar.activation(
                                    pt[:, lo:S], sc[:, lo:S], AF.Exp,
                                    scale=rkt[:, i:i + 1])
                                ptiles.append(pt)

                        # ---- phase B: PV + rowsum + normalize ----
                        with tc.tile_pool(name=f"pv{h}{b}", bufs=2,
                                          space="PSUM") as pvps, \
                             tc.tile_pool(name=f"st{h}{b}", bufs=3) as stp:
                            for j in range(NCH):
                                jmax = 4 * j + 3
                                ot = pvps.tile([128, 512], F32, tag="ot",
                                               name="ot")
                                for i in range(jmax + 1):
                                    off = max(0, 128 * i - 512 * j)
                                    nc.tensor.matmul(
                                        ot[:, off:512],
                                        va[:, 128 * i:128 * (i + 1)],
                                        ptiles[i][:, 512 * j + off:
                                                  512 * (j + 1)],
                                        start=(i == 0), stop=(i == jmax))
                                den = pvps.tile([1, 512], F32, tag="den",
                                                name="den")
                                for i in range(jmax + 1):
                                    off = max(0, 128 * i - 512 * j)
                                    nc.tensor.matmul(
                                        den[:, off:512], ones_col_bf[:],
                                        ptiles[i][:, 512 * j + off:
                                                  512 * (j + 1)],
                                        start=(i == 0), stop=(i == jmax))
                                rec = stp.tile([1, 512], F32R, tag="rec",
                                               name="rec")
                                with nc.allow_low_precision(
                                        reason="softmax denom recip"):
                                    nc.vector.reciprocal(rec[:], den[:])
                                bde = pvps.tile([128, 512], F32, tag="bde",
                                                name="bde")
                                nc.tensor.matmul(bde[:], ones_row[:],
                                                 rec[:], start=True,
                                                 stop=True)
                                bds = stp.tile([128, 512], F32, tag="bds",
                                               name="bds")
                                nc.scalar.copy(bds[:], bde[:])
                                asb = stp.tile([128, 512], BF16, tag="asb",
                                               name="asb")
                                nc.vector.tensor_mul(asb[:], ot[:], bds[:])
                                nc.sync.dma_start(
                                    a2a_in[h][128 * (4 * b + j):
                                              128 * (4 * b + j + 1), :],
                                    asb[:])

                # ---- AllToAll for this head round ----
                nc.gpsimd.collective_compute(
                    "AllToAll", ALU.bypass,
                    replica_groups=[list(range(N_CORES))],
                    ins=[a2a_in[h].opt()], outs=[a2a_out[h].opt()],
                    cc_dim="Partition")

            # ====================== out projection ====================
            with tc.tile_pool(name="opool", bufs=1) as opool, \
                 tc.tile_pool(name="ops", bufs=2, space="PSUM") as ops:
                av = []
                for h in range(2):
                    at = opool.tile([128, 8 * 512], BF16, tag=f"av{h}",
                                    name=f"av{h}")
                    nc.sync.dma_start(
                        at[:].rearrange("p (k s) -> p k s", s=512),
                        a2a_out[h][:].rearrange("(k p) s -> p k s", p=128))
                    av.append(at[:].rearrange("p (k s) -> p k s", s=512))

                for mp in range(4):
                    pso = [ops.tile([128, 512], F32, tag=f"o{n}",
                                    name=f"o{n}") for n in range(NCH)]
                    for g in range(16):
                        stat = av[g % 2][:, g // 2,
                                         128 * mp:128 * (mp + 1)]
                        for n in range(NCH):
                            nc.tensor.matmul(
                                pso[n][:], stat,
                                wov[:, g, 512 * n:512 * (n + 1)],
                                start=(g == 0), stop=(g == 15))
                    for n in range(NCH):
                        os_t = opool.tile([128, 512], F32, tag="osb",
                                          bufs=4, name="os_t")
                        nc.scalar.copy(os_t[:], pso[n][:])
                        nc.sync.dma_start(
                            out.ap()[128 * mp:128 * (mp + 1),
                                     512 * n:512 * (n + 1)], os_t[:])
            wo_cm.__exit__(None, None, None)

    nc.compile()
    return nc


_NC_CACHE = None


def _get_nc():
    global _NC_CACHE
    if _NC_CACHE is None:
        _NC_CACHE = _build()
    return _NC_CACHE


def _host_prep(inputs):
    hs = np.asarray(inputs["hidden_states"], dtype=np.float32)
    Wq = np.asarray(inputs["Wq"], dtype=np.float32)
    Wk = np.asarray(inputs["Wk"], dtype=np.float32)
    Wv = np.asarray(inputs["Wv"], dtype=np.float32)
    Wo = np.asarray(inputs["Wo"], dtype=np.float32)
    cqw = np.asarray(inputs["canon_q_w"], dtype=np.float32)
    ckw = np.asarray(inputs["canon_k_w"], dtype=np.float32)
    cvw = np.asarray(inputs["canon_v_w"], dtype=np.float32)
    qnw = np.asarray(inputs["q_norm_w"], dtype=np.float32)
    knw = np.asarray(inputs["k_norm_w"], dtype=np.float32)

    bf = ml_dtypes.bfloat16
    hsT = np.ascontiguousarray(
        np.concatenate([hs[0].T, hs[1].T], axis=1)).astype(bf)
    WqT, WkT, WvT = Wq.T, Wk.T, Wv.T
    woT = np.ascontiguousarray(Wo.T).astype(bf)

    inv_freq = 1.0 / (10000.0 ** (np.arange(0, DH, 2, dtype=np.float64) / DH))
    freqs = np.arange(S, dtype=np.float64)[:, None] * inv_freq
    emb = np.concatenate([freqs, freqs], axis=-1)
    cosT, sinT = np.cos(emb).T, np.sin(emb).T

    def make_rope(normw, scale):
        A = cosT * normw[:, None] * scale
        wswap = normw[(np.arange(DH) + 64) % DH]
        sign = np.where(np.arange(DH) < 64, -1.0, 1.0)
        Bc = sinT * wswap[:, None] * sign[:, None] * scale
        return (np.ascontiguousarray(A).astype(bf),
                np.ascontiguousarray(Bc).astype(bf))

    Aq, Bq = make_rope(qnw, SCALE)
    Ak, Bk = make_rope(knw, 1.0)

    p = np.arange(128)[:, None]
    f = np.arange(128)[None, :]
    maskd = np.where(p <= f, 0.0, NEG).astype(np.float32)

    in_maps = []
    for r in range(N_CORES):
        wTc = np.ascontiguousarray(np.concatenate(
            [WqT[:, 256 * r:256 * r + 256],
             WkT[:, 128 * r:128 * r + 128],
             WvT[:, 128 * r:128 * r + 128]], axis=1)).astype(bf)
        cwc = np.ascontiguousarray(np.concatenate(
            [cqw[256 * r:256 * r + 256],
             ckw[128 * r:128 * r + 128],
             cvw[128 * r:128 * r + 128]], axis=0)).astype(np.float32)
        in_maps.append({
            "hsT": hsT, "wT": wTc, "woT": woT, "cw": cwc,
            "ropeAq": Aq, "ropeBq": Bq, "ropeAk": Ak, "ropeBk": Bk,
            "maskd": maskd,
        })
    return in_maps


def kernel(**inputs):
    nc = _get_nc()
    in_maps = _host_prep(inputs)
    res = run_bass_kernel_spmd(nc, in_maps, core_ids=list(range(N_CORES)))
    full = np.empty((B, S, D), np.float32)
    for r in range(N_CORES):
        full[r // 4, 512 * (r % 4):512 * (r % 4 + 1), :] = res.results[r]["out"]
    return full


# revision 18
# speedup vs baseline: 1.1182x; 1.1182x over previous
"""Trainium2 Bass kernel for CanonCausalMultiheadAttn (v3).

Sharding: tensor-parallel over heads across 8 cores (2 q-heads + 1 kv-head
per core), both batches replicated. Attention outputs are exchanged with
TWO AllToAlls (one per local q-head) so round 0 overlaps round-1 compute;
each core then owns one (batch, 512-seq-slice) of the output projection.

Per-core pipeline (B=2, S=2048, D=2048 hardcoded):
  QKV proj: bf16 matmuls, the PE queue is pure matmuls plus tiny norm
  reductions -> canon conv on full bf16 rows (DVE taps at 2x 16-bit rate,
  first tap folded into the ACT psum-drain copy) -> qk rmsnorm:
  sum-of-squares via ones-column matmuls, q-rstd broadcast via ones-row
  matmul + wide 128-lane reciprocal (no single-partition reciprocals
  anywhere), k-rstd transposed via DRAM and applied as the EXP
  per-partition scale -> RoPE (bf16 DVE, norm weight & 1/sqrt(dh) folded
  into bf16 tables) -> causal attention with scores in [Sk, Sq] layout;
  per K-block one wide matmul row + one wide EXP -> PV swapped
  (stationary = V block, moving = P) emitting O'^T [dh, q] directly in
  a2a layout, denominators via ones-column rowsum matmuls, normalization
  via broadcast matmul + wide reciprocal folded into the psum drain ->
  AllToAll x2 -> output projection (stationary = received attn block,
  resident Wo in SBUF). DMA triggers are spread across engine queues
  (weights/activations on Sync, V transposes on GpSimd, rope shift on
  Vector, norm roundtrip + tables on Scalar) to avoid head-of-line
  blocking of the weight stream.
"""
import sys

sys.path.insert(0, '/opt/trn_rl_repo')

import numpy as np
import ml_dtypes

import concourse.bass as bass
import concourse.mybir as mybir
import concourse.tile as tile
from concourse import bacc
from concourse.bass_utils import run_bass_kernel_spmd

F32 = mybir.dt.float32
F32R = mybir.dt.float32r
BF16 = mybir.dt.bfloat16
AF = mybir.ActivationFunctionType
ALU = mybir.AluOpType

B, S, D = 2, 2048, 2048
NH, NKV, DH = 16, 8, 128
K_CONV = 4
EPS = 1e-6
SCALE = 1.0 / float(np.sqrt(DH))
NEG = -1e9
N_CORES = 8
NCH = S // 512          # 512-wide seq chunks
NI = S // 128           # 128-wide Sk blocks


def _build():
    nc = bacc.Bacc("TRN2", target_bir_lowering=False, debug=False,
                   num_devices=N_CORES)

    hsT = nc.dram_tensor("hsT", [D, B * S], BF16, kind="ExternalInput")
    wT = nc.dram_tensor("wT", [D, 512], BF16, kind="ExternalInput")
    woT = nc.dram_tensor("woT", [D, D], BF16, kind="ExternalInput")
    cw = nc.dram_tensor("cw", [512, K_CONV], F32, kind="ExternalInput")
    ropeAq = nc.dram_tensor("ropeAq", [DH, S], BF16, kind="ExternalInput")
    ropeBq = nc.dram_tensor("ropeBq", [DH, S], BF16, kind="ExternalInput")
    ropeAk = nc.dram_tensor("ropeAk", [DH, S], BF16, kind="ExternalInput")
    ropeBk = nc.dram_tensor("ropeBk", [DH, S], BF16, kind="ExternalInput")
    maskd = nc.dram_tensor("maskd", [128, 128], F32, kind="ExternalInput")
    out = nc.dram_tensor("out", [512, D], F32, kind="ExternalOutput")

    with tile.TileContext(nc) as tc:
        with tc.tile_pool(name="const", bufs=1) as cpool, \
             tc.tile_pool(name="pers", bufs=1) as pers, \
             tc.tile_pool(name="dram", bufs=1, space="DRAM") as dram:

            # ---- small constants (triggers on scalar queue; sync stays
            # free for the weight/activation stream) ----
            cw_sb = []
            for mt in range(4):
                t = cpool.tile([128, K_CONV], F32, tag=f"cw{mt}",
                               name=f"cw{mt}")
                nc.scalar.dma_start(t[:], cw.ap()[128 * mt:128 * mt + 128, :])
                cw_sb.append(t)
            mask_sb = cpool.tile([128, 128], F32, tag="mask")
            nc.scalar.dma_start(mask_sb[:], maskd.ap())
            ones_col_f = cpool.tile([128, 1], F32, tag="ocf")
            nc.vector.memset(ones_col_f[:], 1.0)
            ones_row_f = cpool.tile([1, 128], F32, tag="orf")
            nc.vector.memset(ones_row_f[:], 1.0)
            ones_row = cpool.tile([1, 128], F32R, tag="or")
            nc.scalar.copy(ones_row[:], ones_row_f[:])
            ones_col_bf = cpool.tile([128, 1], BF16, tag="ocb")
            nc.scalar.copy(ones_col_bf[:], ones_col_f[:])
            eps_sb = cpool.tile([1, 1], F32, tag="eps")
            nc.vector.memset(eps_sb[:], EPS)
            s0_sb = []
            for mt in range(4):
                t = cpool.tile([128, 1], F32, tag=f"s0{mt}", name=f"s0{mt}")
                nc.vector.tensor_scalar_add(t[:], cw_sb[mt][:, 0:1], 1.0)
                s0_sb.append(t)

            # persistent per-batch tiles
            roped = {}   # (b, mt<3) -> [128, S] bf16   (q0, q1, kT)
            vaug = {}    # b -> [128, NI*128] bf16      (V transposed blocks)
            rstdkT = {}  # b -> [128, NI] f32

            for b in range(B):
                for mt in range(3):
                    roped[(b, mt)] = pers.tile([128, S], BF16,
                                               tag=f"roped{b}{mt}",
                                               name=f"roped{b}{mt}")
                vaug[b] = pers.tile([128, NI * 128], BF16, tag=f"vaug{b}",
                                    name=f"vaug{b}")
                rstdkT[b] = pers.tile([128, NI], F32, tag=f"rstdkT{b}",
                                      name=f"rstdkT{b}")

            srt_d = {b: dram.tile([NI, 128], F32, tag=f"srtd{b}",
                                  name=f"srt_d{b}") for b in range(B)}

            # rope tables (bf16), freed after prep
            with tc.tile_pool(name="ropes", bufs=1) as rpool:
                ropes = {}
                for nm, t in (("Aq", ropeAq), ("Bq", ropeBq),
                              ("Ak", ropeAk), ("Bk", ropeBk)):
                    rt = rpool.tile([DH, S], BF16, tag=f"rope{nm}",
                                    name=f"rope{nm}")
                    nc.scalar.dma_start(rt[:], t.ap())
                    ropes[nm] = rt

                hs_cm = tc.tile_pool(name="hs", bufs=1)
                hs_pool = hs_cm.__enter__()
                hs_sb = hs_pool.tile([128, 16 * S], BF16, tag="hs",
                                     name="hs_sb")
                hv = hs_sb[:].rearrange("p (k s) -> p k s", s=S)

                for b in range(B):
                    # =============== QKV projection, batch b ===============
                    for kg in range(8):
                        nc.sync.dma_start(
                            hv[:, 2 * kg:2 * kg + 2, :],
                            hsT.ap()[:, b * S:(b + 1) * S]
                            .rearrange("(k p) s -> p k s", p=128)
                            [:, 2 * kg:2 * kg + 2, :])

                    with tc.tile_pool(name=f"cn{b}", bufs=1) as bw:
                        cn = {}
                        for mt in range(4):
                            cn[mt] = bw.tile([128, S], BF16, tag=f"cn{mt}",
                                             name=f"cn{mt}")

                        with tc.tile_pool(name=f"qps{b}", bufs=2,
                                          space="PSUM") as qps:
                            for mt in range(4):
                                psums = [qps.tile([128, 512], F32,
                                                  tag=f"q{n}", name=f"q{n}")
                                         for n in range(NCH)]
                                for k in range(16):
                                    wt_k = bw.tile([128, 128], BF16,
                                                   tag="wtk", bufs=6,
                                                   name="wt_k")
                                    nc.sync.dma_start(
                                        wt_k[:],
                                        wT.ap()[128 * k:128 * (k + 1),
                                                128 * mt:128 * (mt + 1)])
                                    for n in range(NCH):
                                        nc.tensor.matmul(
                                            psums[n][:], wt_k[:],
                                            hv[:, k, 512 * n:512 * (n + 1)],
                                            start=(k == 0), stop=(k == 15))
                                # drain: raw (scale 1) + cn (scale 1+w0)
                                raw = bw.tile([128, S], BF16, tag="raw",
                                              bufs=2, name="raw")
                                c = cn[mt]
                                for n in range(NCH):
                                    sl = slice(512 * n, 512 * (n + 1))
                                    nc.scalar.copy(raw[:, sl], psums[n][:])
                                    nc.scalar.activation(
                                        c[:, sl], psums[n][:], AF.Copy,
                                        scale=s0_sb[mt][:])
                                # canon taps on the full row (in-place)
                                for k in range(1, K_CONV):
                                    nc.vector.scalar_tensor_tensor(
                                        c[:, k:S], raw[:, 0:S - k],
                                        cw_sb[mt][:, k:k + 1], c[:, k:S],
                                        ALU.mult, ALU.add)

                        # =============== prep, batch b =====================
                        # V: transpose canon output into va blocks (gpsimd q)
                        va = vaug[b]
                        for i in range(NI):
                            nc.scalar.dma_start_transpose(
                                va[:, 128 * i:128 * (i + 1)],
                                cn[3][:, 128 * i:128 * (i + 1)])

                        # sum of squares + rstd
                        sq = {}
                        for mt in range(3):
                            sq[mt] = bw.tile([128, S], BF16, tag=f"sq{mt}",
                                             name=f"sq{mt}")
                            nc.vector.tensor_mul(sq[mt][:], cn[mt][:],
                                                 cn[mt][:])
                        bcq = {}
                        with tc.tile_pool(name=f"nps{b}", bufs=2,
                                          space="PSUM") as nps, \
                             tc.tile_pool(name=f"bps{b}", bufs=2,
                                          space="PSUM") as bps:
                            srtk = bw.tile([1, S], F32, tag="srtk",
                                           name="srtk")
                            for mt in range(3):
                                is_q = mt < 2
                                if is_q:
                                    bc = bw.tile([128, S], BF16,
                                                 tag=f"bc{mt}", name="bc")
                                    bcq[mt] = bc
                                for cch in range(NCH):
                                    sl = slice(512 * cch, 512 * (cch + 1))
                                    sp = nps.tile([1, 512], F32, tag="ssq")
                                    nc.tensor.matmul(sp[:], ones_col_bf[:],
                                                     sq[mt][:, sl],
                                                     start=True, stop=True)
                                    if is_q:
                                        srt = bw.tile([1, 512], F32R,
                                                      tag="srtq", bufs=2,
                                                      name="srt")
                                        nc.scalar.activation(
                                            srt[:], sp[:], AF.Sqrt,
                                            bias=eps_sb[:], scale=1.0 / DH)
                                        bp = bps.tile([128, 512], F32,
                                                      tag="bcp")
                                        nc.tensor.matmul(bp[:],
                                                         ones_row[:],
                                                         srt[:],
                                                         start=True,
                                                         stop=True)
                                        with nc.allow_low_precision(
                                                reason="rstd bf16 ample"):
                                            nc.vector.reciprocal(
                                                bc[:, sl], bp[:])
                                    else:
                                        nc.scalar.activation(
                                            srtk[:, sl], sp[:], AF.Sqrt,
                                            bias=eps_sb[:], scale=1.0 / DH)
                            # k: transpose srt via DRAM, then wide recip
                            nc.scalar.dma_start(srt_d[b][:], srtk[:])
                            srtkT = bw.tile([128, NI], F32, tag="srtkT",
                                            name="srtkT")
                            nc.scalar.dma_start(
                                srtkT[:], srt_d[b][:].rearrange("i p -> p i"))
                            nc.vector.reciprocal(rstdkT[b][:], srtkT[:])

                        # rope
                        for mt in range(3):
                            x = cn[mt]
                            is_q = mt < 2
                            A_ = ropes["Aq"] if is_q else ropes["Ak"]
                            B_ = ropes["Bq"] if is_q else ropes["Bk"]
                            sh = bw.tile([128, S], BF16, tag="sh", bufs=2,
                                         name="sh")
                            nc.gpsimd.dma_start(sh[0:64, :], x[64:128, :])
                            nc.gpsimd.dma_start(sh[64:128, :], x[0:64, :])
                            nc.vector.tensor_mul(sh[:], sh[:], B_[:])
                            tm = bw.tile([128, S], BF16, tag="tm", bufs=2,
                                         name="tm")
                            nc.vector.tensor_mul(tm[:], x[:], A_[:])
                            ro = roped[(b, mt)]
                            if is_q:
                                nc.vector.tensor_add(tm[:], tm[:], sh[:])
                                nc.vector.tensor_mul(ro[:], tm[:],
                                                     bcq[mt][:])
                            else:
                                nc.vector.tensor_add(ro[:], tm[:], sh[:])

                hs_cm.__exit__(None, None, None)

            # ======================= attention =======================
            a2a_in = [dram.tile([1024, 512], BF16, tag=f"a2ai{h}",
                                name=f"a2a_in{h}") for h in range(2)]
            a2a_out = [dram.tile([1024, 512], BF16, tag=f"a2ao{h}",
                                 name=f"a2a_out{h}") for h in range(2)]

            # resident Wo, prefetched during attention
            wo_cm = tc.tile_pool(name="wo", bufs=1)
            wo_pool = wo_cm.__enter__()
            wo_sb = wo_pool.tile([128, 16 * D], BF16, tag="wo",
                                 name="wo_sb")
            wov = wo_sb[:].rearrange("p (k c) -> p k c", c=D)
            for kg in range(4):
                nc.sync.dma_start(
                    wov[:, 4 * kg:4 * kg + 4, :],
                    woT.ap().rearrange("(k p) c -> p k c", p=128)
                    [:, 4 * kg:4 * kg + 4, :])

            for h in range(2):
                for b in range(B):
                    KT = roped[(b, 2)]
                    QT = roped[(b, h)]
                    va = vaug[b]
                    rkt = rstdkT[b]
                    with tc.tile_pool(name=f"pp{h}{b}", bufs=1) as ppool:
                        ptiles = []
                        # ---- phase A: scores + exp, per Sk block ----
                        with tc.tile_pool(name=f"sc{h}{b}", bufs=2,
                                          space="PSUM") as scps:
                            for i in range(NI):
                                lo = 128 * i
                                sc = scps.tile([128, 2048], F32, tag="sc",
                                               name="sc")
                                for n in range(lo // 512, NCH):
                                    c0 = max(lo, 512 * n)
                                    nc.tensor.matmul(
                                        sc[:, c0:512 * (n + 1)],
                                        KT[:, lo:lo + 128],
                                        QT[:, c0:512 * (n + 1)],
                                        start=True, stop=True)
                                nc.vector.tensor_add(
                                    sc[:, lo:lo + 128],
                                    sc[:, lo:lo + 128], mask_sb[:])
                                pt = ppool.tile([128, 2048], BF16,
                                                tag=f"p{i}", name=f"p{i}")
                                nc.scalar.activation(
                                    pt[:, lo:S], sc[:, lo:S], AF.Exp,
                                    scale=rkt[:, i:i + 1])
                                ptiles.append(pt)

                        # ---- phase B: PV + rowsum, then normalize ----
                        with tc.tile_pool(name=f"pv{h}{b}", bufs=1,
                                          space="PSUM") as pvps, \
                             tc.tile_pool(name=f"st{h}{b}", bufs=2) as stp:
                            ots, dens = [], []
                            for j in range(NCH):
                                jmax = 4 * j + 3
                                ot = pvps.tile([128, 512], F32,
                                               tag=f"ot{j}", name=f"ot{j}")
                                for i in range(jmax + 1):
                                    off = max(0, 128 * i - 512 * j)
                                    nc.tensor.matmul(
                                        ot[:, off:512],
                                        va[:, 128 * i:128 * (i + 1)],
                                        ptiles[i][:, 512 * j + off:
                                                  512 * (j + 1)],
                                        start=(i == 0), stop=(i == jmax))
                                den = pvps.tile([1, 512], F32, tag="dn",
                                                bufs=2, name="den")
                                for i in range(jmax + 1):
                                    off = max(0, 128 * i - 512 * j)
                                    nc.tensor.matmul(
                                        den[:, off:512], ones_col_bf[:],
                                        ptiles[i][:, 512 * j + off:
                                                  512 * (j + 1)],
                                        start=(i == 0), stop=(i == jmax))
                                dsb = stp.tile([1, 512], F32R, tag="dsb",
                                               bufs=2, name="dsb")
                                nc.scalar.copy(dsb[:], den[:])
                                ots.append(ot)
                                dens.append(dsb)
                            for j in range(NCH):
                                bde = pvps.tile([128, 512], F32, tag="bd",
                                                bufs=2, name="bde")
                                nc.tensor.matmul(bde[:], ones_row[:],
                                                 dens[j][:], start=True,
                                                 stop=True)
                                brc = stp.tile([128, 512], F32, tag="brc",
                                               bufs=2, name="brc")
                                nc.vector.reciprocal(brc[:], bde[:])
                                asb = stp.tile([128, 512], BF16, tag="asb",
                                               bufs=2, name="asb")
                                nc.vector.tensor_mul(asb[:], ots[j][:],
                                                     brc[:])
                                nc.sync.dma_start(
                                    a2a_in[h][128 * (4 * b + j):
                                              128 * (4 * b + j + 1), :],
                                    asb[:])

                # ---- AllToAll for this head round ----
                nc.gpsimd.collective_compute(
                    "AllToAll", ALU.bypass,
                    replica_groups=[list(range(N_CORES))],
                    ins=[a2a_in[h].opt()], outs=[a2a_out[h].opt()],
                    cc_dim="Partition")

            # ====================== out projection ====================
            with tc.tile_pool(name="opool", bufs=1) as opool, \
                 tc.tile_pool(name="ops", bufs=2, space="PSUM") as ops:
                av = []
                for h in range(2):
                    at = opool.tile([128, 8 * 512], BF16, tag=f"av{h}",
                                    name=f"av{h}")
                    nc.sync.dma_start(
                        at[:].rearrange("p (k s) -> p k s", s=512),
                        a2a_out[h][:].rearrange("(k p) s -> p k s", p=128))
                    av.append(at[:].rearrange("p (k s) -> p k s", s=512))

                for mp in range(4):
                    pso = [ops.tile([128, 512], F32, tag=f"o{n}",
                                    name=f"o{n}") for n in range(NCH)]
                    for g in range(16):
                        stat = av[g % 2][:, g // 2,
                                         128 * mp:128 * (mp + 1)]
                        for n in range(NCH):
                            nc.tensor.matmul(
                                pso[n][:], stat,
                                wov[:, g, 512 * n:512 * (n + 1)],
                                start=(g == 0), stop=(g == 15))
                    for n in range(NCH):
                        os_t = opool.tile([128, 512], F32, tag="osb",
                                          bufs=4, name="os_t")
                        nc.scalar.copy(os_t[:], pso[n][:])
                        nc.sync.dma_start(
                            out.ap()[128 * mp:128 * (mp + 1),
                                     512 * n:512 * (n + 1)], os_t[:])
            wo_cm.__exit__(None, None, None)

    nc.compile()
    return nc


_NC_CACHE = None


def _get_nc():
    global _NC_CACHE
    if _NC_CACHE is None:
        _NC_CACHE = _build()
    return _NC_CACHE


def _host_prep(inputs):
    hs = np.asarray(inputs["hidden_states"], dtype=np.float32)
    Wq = np.asarray(inputs["Wq"], dtype=np.float32)
    Wk = np.asarray(inputs["Wk"], dtype=np.float32)
    Wv = np.asarray(inputs["Wv"], dtype=np.float32)
    Wo = np.asarray(inputs["Wo"], dtype=np.float32)
    cqw = np.asarray(inputs["canon_q_w"], dtype=np.float32)
    ckw = np.asarray(inputs["canon_k_w"], dtype=np.float32)
    cvw = np.asarray(inputs["canon_v_w"], dtype=np.float32)
    qnw = np.asarray(inputs["q_norm_w"], dtype=np.float32)
    knw = np.asarray(inputs["k_norm_w"], dtype=np.float32)

    bf = ml_dtypes.bfloat16
    hsT = np.ascontiguousarray(
        np.concatenate([hs[0].T, hs[1].T], axis=1)).astype(bf)
    WqT, WkT, WvT = Wq.T, Wk.T, Wv.T
    woT = np.ascontiguousarray(Wo.T).astype(bf)

    inv_freq = 1.0 / (10000.0 ** (np.arange(0, DH, 2, dtype=np.float64) / DH))
    freqs = np.arange(S, dtype=np.float64)[:, None] * inv_freq
    emb = np.concatenate([freqs, freqs], axis=-1)
    cosT, sinT = np.cos(emb).T, np.sin(emb).T

    def make_rope(normw, scale):
        A = cosT * normw[:, None] * scale
        wswap = normw[(np.arange(DH) + 64) % DH]
        sign = np.where(np.arange(DH) < 64, -1.0, 1.0)
        Bc = sinT * wswap[:, None] * sign[:, None] * scale
        return (np.ascontiguousarray(A).astype(bf),
                np.ascontiguousarray(Bc).astype(bf))

    Aq, Bq = make_rope(qnw, SCALE)
    Ak, Bk = make_rope(knw, 1.0)

    p = np.arange(128)[:, None]
    f = np.arange(128)[None, :]
    maskd = np.where(p <= f, 0.0, NEG).astype(np.float32)

    in_maps = []
    for r in range(N_CORES):
        wTc = np.ascontiguousarray(np.concatenate(
            [WqT[:, 256 * r:256 * r + 256],
             WkT[:, 128 * r:128 * r + 128],
             WvT[:, 128 * r:128 * r + 128]], axis=1)).astype(bf)
        cwc = np.ascontiguousarray(np.concatenate(
            [cqw[256 * r:256 * r + 256],
             ckw[128 * r:128 * r + 128],
             cvw[128 * r:128 * r + 128]], axis=0)).astype(np.float32)
        in_maps.append({
            "hsT": hsT, "wT": wTc, "woT": woT, "cw": cwc,
            "ropeAq": Aq, "ropeBq": Bq, "ropeAk": Ak, "ropeBk": Bk,
            "maskd": maskd,
        })
    return in_maps


def kernel(**inputs):
    nc = _get_nc()
    in_maps = _host_prep(inputs)
    res = run_bass_kernel_spmd(nc, in_maps, core_ids=list(range(N_CORES)))
    full = np.empty((B, S, D), np.float32)
    for r in range(N_CORES):
        full[r // 4, 512 * (r % 4):512 * (r % 4 + 1), :] = res.results[r]["out"]
    return full
